# revision 5
# baseline (speedup 1.0000x reference)
"""LipschitzRNN Trainium2 kernel.

Math (per reference):
    bA = 0.5*exp(-bA_z^2)+0.5 ; bW likewise
    A = (1-bA)(MA+MA.T) + bA(MA-MA.T) - YA*I
    C = (1-bA)(MW+MW.T) + bW(MW-MW.T) - YW*I
    X_{t+1} = X_t + STEP*(A@X_t + tanh(C@X_t + by))   (column-state X: [n, bs])
    out[b, t, :] = X_t[:, b]

Device strategy (8-way batch data-parallel, b=32/core):
  - State kept as [n(partitions), b] in SBUF: two k-chunks side by side [128, 64].
  - Weights stationary per matmul ([A-fold; C] in lhsT [k, m] layout), float32r
    (fp22 multiplies, fp32 accumulate) self-loading matmuls.
  - Folded form: G = (I + STEP*A)@X via WG, V = C@X via WC;
    tanh fused with per-partition bias `by` on ScalarE;
    X_next = (tanh * STEP) + G in one VectorE scalar_tensor_tensor.
  - Output needs [b, n] rows: PE transpose each new state, DVE copy to SBUF,
    DMA to OUT[b, t, :].
"""

import os
import numpy as np

N = 256
BS = 256
TMAX = 512
STEP = 0.01
YA = 0.001
YW = 0.001
NCORES = 8
BLOC = BS // NCORES  # 32

LAST_RESULT = None  # BassKernelResults of the most recent run (for test harness)


def _build(n_steps):
    from concourse import bacc, tile
    import concourse.mybir as mybir
    from concourse.masks import make_identity

    F32 = mybir.dt.float32
    F32R = mybir.dt.float32r
    AF = mybir.ActivationFunctionType
    ALU = mybir.AluOpType

    nc = bacc.Bacc("TRN2", target_bir_lowering=False, debug=False,
                   num_devices=NCORES)

    WA = nc.dram_tensor("WA", [N, N], F32R, kind="ExternalInput")    # A.T  [k, m], fp22-rounded
    WC = nc.dram_tensor("WC", [N, N], F32R, kind="ExternalInput")    # C.T  [k, m], fp22-rounded
    BY = nc.dram_tensor("BY", [N, 1], F32, kind="ExternalInput")
    X0T = nc.dram_tensor("X0T", [N, BLOC], F32, kind="ExternalInput")
    OUT = nc.dram_tensor("OUT", [BLOC, TMAX, N], F32, kind="ExternalOutput")

    with tile.TileContext(nc) as tc:
        with (
            tc.tile_pool(name="consts", bufs=1) as consts,
            tc.tile_pool(name="xqpool", bufs=3) as xqpool,
            tc.tile_pool(name="mpool", bufs=3) as mpool,
            tc.tile_pool(name="ppool", bufs=2) as ppool,
            tc.tile_pool(name="tpool", bufs=2) as tpool,
            tc.tile_pool(name="stpool", bufs=4) as stpool,
            tc.tile_pool(name="psv", bufs=2, space="PSUM") as psv,
            tc.tile_pool(name="psu", bufs=2, space="PSUM") as psu,
            tc.tile_pool(name="pst", bufs=3, space="PSUM") as pst,
        ):
            # ---- constants / initial state ----
            wa = [consts.tile([128, N], F32R, name=f"wa{k}", tag=f"wa{k}") for k in range(2)]
            wc = [consts.tile([128, N], F32R, name=f"wc{k}", tag=f"wc{k}") for k in range(2)]
            for k in range(2):
                nc.sync.dma_start(wa[k][:], WA[128 * k:128 * (k + 1), :])
                nc.sync.dma_start(wc[k][:], WC[128 * k:128 * (k + 1), :])
            by_sb = consts.tile([128, 2], F32, tag="by")
            nc.sync.dma_start(by_sb[:, 0:1], BY[0:128, :])
            nc.sync.dma_start(by_sb[:, 1:2], BY[128:256, :])
            ident = consts.tile([128, 128], F32, tag="ident")
            make_identity(nc, ident[:])

            m = mpool.tile([128, 2 * BLOC], F32, tag="m")   # master fp32 state
            nc.sync.dma_start(m[:, 0:BLOC], X0T[0:128, :])
            nc.sync.dma_start(m[:, BLOC:2 * BLOC], X0T[128:256, :])
            xq = xqpool.tile([128, 2 * BLOC], F32R, tag="xq")  # fp22 copy for PE
            nc.vector.tensor_copy(xq[:], m[:])

            # ---- recurrence: M_i = M_{i-1} + STEP*(A@xq + tanh(C@xq + by)) ----
            for t in range(1, n_steps + 1):
                pv = psv.tile([128, 2 * BLOC], F32, tag="pv")
                pu = psu.tile([128, 2 * BLOC], F32, tag="pu")
                # V = C@xq first (feeds the tanh -> chain), then U = A@xq
                for mc in range(2):
                    ms = slice(BLOC * mc, BLOC * (mc + 1))
                    nc.tensor.matmul(pv[:, ms], wc[0][:, 128 * mc:128 * (mc + 1)],
                                     xq[:, 0:BLOC], start=True, stop=False)
                    nc.tensor.matmul(pv[:, ms], wc[1][:, 128 * mc:128 * (mc + 1)],
                                     xq[:, BLOC:2 * BLOC], start=False, stop=True)
                for mc in range(2):
                    ms = slice(BLOC * mc, BLOC * (mc + 1))
                    nc.tensor.matmul(pu[:, ms], wa[0][:, 128 * mc:128 * (mc + 1)],
                                     xq[:, 0:BLOC], start=True, stop=False)
                    nc.tensor.matmul(pu[:, ms], wa[1][:, 128 * mc:128 * (mc + 1)],
                                     xq[:, BLOC:2 * BLOC], start=False, stop=True)

                # P = M + STEP*U  (off the tanh chain; runs while ACT computes tanh)
                p = ppool.tile([128, 2 * BLOC], F32, tag="p")
                nc.vector.scalar_tensor_tensor(
                    p[:], pu[:], STEP, m[:], op0=ALU.mult, op1=ALU.add)

                tt = tpool.tile([128, 2 * BLOC], F32, tag="tt")
                nc.scalar.activation(tt[:, 0:BLOC], pv[:, 0:BLOC], AF.Tanh,
                                     bias=by_sb[:, 0:1], scale=1.0)
                nc.scalar.activation(tt[:, BLOC:2 * BLOC], pv[:, BLOC:2 * BLOC],
                                     AF.Tanh, bias=by_sb[:, 1:2], scale=1.0)

                # chain op: next PE input (fp22-rounded by f32r output dtype)
                xq = xqpool.tile([128, 2 * BLOC], F32R, tag="xq")
                nc.vector.scalar_tensor_tensor(
                    xq[:], tt[:], STEP, p[:], op0=ALU.mult, op1=ALU.add)
                # master state, same math in fp32 (off chain)
                m = mpool.tile([128, 2 * BLOC], F32, tag="m")
                nc.vector.scalar_tensor_tensor(
                    m[:], tt[:], STEP, p[:], op0=ALU.mult, op1=ALU.add)

                # output row t: transpose master state [128, 2b] -> [b, 256]
                pt = pst.tile([BLOC, N], F32, tag="pt")
                nc.tensor.transpose(pt[:, 0:128], m[:, 0:BLOC], ident[:])
                nc.tensor.transpose(pt[:, 128:256], m[:, BLOC:2 * BLOC], ident[:])
                stage = stpool.tile([BLOC, N], F32, tag="stage")
                nc.vector.tensor_copy(stage[:], pt[:])
                nc.sync.dma_start(OUT[:, t, :], stage[:])
    nc.compile()
    return nc


def kernel(X0, MA, MW, bA_z, bW_z, by_w):
    global LAST_RESULT
    from concourse.bass_utils import run_bass_kernel_spmd

    X0 = np.asarray(X0, dtype=np.float32)
    MA = np.asarray(MA, dtype=np.float32)
    MW = np.asarray(MW, dtype=np.float32)
    bA_z = np.asarray(bA_z, dtype=np.float32)
    bW_z = np.asarray(bW_z, dtype=np.float32)
    by_w = np.asarray(by_w, dtype=np.float32)

    # host-side weight prep (f32, matches reference math); weights pre-rounded
    # to nearest fp22 so the PE's truncate-on-read is exact.
    def round_fp22(x):
        xi = np.ascontiguousarray(x, dtype=np.float32).view(np.uint32)
        xi = (xi + np.uint32(0x200)) & np.uint32(0xFFFFFC00)
        return xi.view(np.float32)

    bA = np.float32(0.5) * np.exp(-bA_z[0, 0] * bA_z[0, 0]) + np.float32(0.5)
    bW = np.float32(0.5) * np.exp(-bW_z[0, 0] * bW_z[0, 0]) + np.float32(0.5)
    I = np.eye(N, dtype=np.float32)
    A = (1 - bA) * (MA + MA.T) + bA * (MA - MA.T) - np.float32(YA) * I
    C = (1 - bA) * (MW + MW.T) + bW * (MW - MW.T) - np.float32(YW) * I
    WA = round_fp22(A.T)
    WC = round_fp22(C.T)

    n_steps = TMAX - 1
    in_maps = []
    for i in range(NCORES):
        in_maps.append({
            "WA": WA,
            "WC": WC,
            "BY": by_w,
            "X0T": np.ascontiguousarray(X0[i * BLOC:(i + 1) * BLOC, :].T),
        })

    nc = _build(n_steps)
    res = run_bass_kernel_spmd(nc, in_maps, core_ids=list(range(NCORES)))
    LAST_RESULT = res

    out = np.concatenate([r["OUT"] for r in res.results], axis=0)
    out[:, 0, :] = X0
    return out


if __name__ == "__main__":
    rng = np.random.default_rng(0)
    inputs = {
        "X0": rng.standard_normal((BS, N), dtype=np.float32),
        "MA": rng.standard_normal((N, N), dtype=np.float32) / 16,
        "MW": rng.standard_normal((N, N), dtype=np.float32) / 16,
        "bA_z": np.full((1, 1), 0.65, dtype=np.float32),
        "bW_z": np.full((1, 1), 0.65, dtype=np.float32),
        "by_w": rng.standard_normal((N, 1), dtype=np.float32) / 100,
    }
    out = kernel(**inputs)
    print("out", out.shape, out.dtype, np.abs(out).max())


# revision 7
# speedup vs baseline: 2.0240x; 2.0240x over previous
"""LipschitzRNN Trainium2 kernel.

Math (per reference):
    bA = 0.5*exp(-bA_z^2)+0.5 ; bW likewise
    A = (1-bA)(MA+MA.T) + bA(MA-MA.T) - YA*I
    C = (1-bA)(MW+MW.T) + bW(MW-MW.T) - YW*I
    X_{t+1} = X_t + STEP*(A@X_t + tanh(C@X_t + by))   (column-state X: [n, bs])
    out[b, t, :] = X_t[:, b]

Device strategy (8-way batch data-parallel, b=32/core):
  - State kept as [n(partitions), b] in SBUF: two k-chunks side by side [128, 64].
  - Weights stationary per matmul ([A-fold; C] in lhsT [k, m] layout), float32r
    (fp22 multiplies, fp32 accumulate) self-loading matmuls.
  - Folded form: G = (I + STEP*A)@X via WG, V = C@X via WC;
    tanh fused with per-partition bias `by` on ScalarE;
    X_next = (tanh * STEP) + G in one VectorE scalar_tensor_tensor.
  - Output needs [b, n] rows: PE transpose each new state, DVE copy to SBUF,
    DMA to OUT[b, t, :].
"""

import os
import numpy as np

N = 256
BS = 256
TMAX = 512
STEP = 0.01
YA = 0.001
YW = 0.001
NCORES = 8
BLOC = BS // NCORES  # 32

LAST_RESULT = None  # BassKernelResults of the most recent run (for test harness)


def _build(n_steps):
    from concourse import bacc, tile
    import concourse.mybir as mybir
    from concourse.masks import make_identity

    F32 = mybir.dt.float32
    F16 = mybir.dt.float16
    AF = mybir.ActivationFunctionType
    ALU = mybir.AluOpType

    nc = bacc.Bacc("TRN2", target_bir_lowering=False, debug=False,
                   num_devices=NCORES)

    WA = nc.dram_tensor("WA", [N, N], F16, kind="ExternalInput")    # A.T  [k, m]
    WC = nc.dram_tensor("WC", [N, N], F16, kind="ExternalInput")    # C.T  [k, m]
    BY = nc.dram_tensor("BY", [N, 1], F32, kind="ExternalInput")
    X0T = nc.dram_tensor("X0T", [N, BLOC], F32, kind="ExternalInput")
    OUT = nc.dram_tensor("OUT", [BLOC, TMAX, N], F32, kind="ExternalOutput")

    with tile.TileContext(nc) as tc:
        with (
            tc.tile_pool(name="consts", bufs=1) as consts,
            tc.tile_pool(name="xqpool", bufs=3) as xqpool,
            tc.tile_pool(name="mpool", bufs=3) as mpool,
            tc.tile_pool(name="ppool", bufs=2) as ppool,
            tc.tile_pool(name="tpool", bufs=2) as tpool,
            tc.tile_pool(name="stpool", bufs=4) as stpool,
            tc.tile_pool(name="psv", bufs=2, space="PSUM") as psv,
            tc.tile_pool(name="psu", bufs=2, space="PSUM") as psu,
            tc.tile_pool(name="pst", bufs=3, space="PSUM") as pst,
        ):
            # ---- constants / initial state ----
            wa = [consts.tile([128, N], F16, name=f"wa{k}", tag=f"wa{k}") for k in range(2)]
            wc = [consts.tile([128, N], F16, name=f"wc{k}", tag=f"wc{k}") for k in range(2)]
            for k in range(2):
                nc.sync.dma_start(wa[k][:], WA[128 * k:128 * (k + 1), :])
                nc.sync.dma_start(wc[k][:], WC[128 * k:128 * (k + 1), :])
            by_sb = consts.tile([128, 2], F32, tag="by")
            nc.sync.dma_start(by_sb[:, 0:1], BY[0:128, :])
            nc.sync.dma_start(by_sb[:, 1:2], BY[128:256, :])
            ident_f32 = consts.tile([128, 128], F32, tag="ident_f32")
            make_identity(nc, ident_f32[:])
            ident = consts.tile([128, 128], F16, tag="ident")
            nc.vector.tensor_copy(ident[:], ident_f32[:])

            m = mpool.tile([128, 2 * BLOC], F32, tag="m")   # master fp32 state
            nc.sync.dma_start(m[:, 0:BLOC], X0T[0:128, :])
            nc.sync.dma_start(m[:, BLOC:2 * BLOC], X0T[128:256, :])
            xq = xqpool.tile([128, 2 * BLOC], F16, tag="xq")  # fp16 copy for PE
            nc.vector.tensor_copy(xq[:], m[:])

            # ---- recurrence: M_i = M_{i-1} + STEP*(A@xq + tanh(C@xq + by)) ----
            for t in range(1, n_steps + 1):
                pv = psv.tile([128, 2 * BLOC], F32, tag="pv")
                pu = psu.tile([128, 2 * BLOC], F32, tag="pu")
                # V = C@xq first (feeds the tanh -> chain), then U = A@xq
                for mc in range(2):
                    ms = slice(BLOC * mc, BLOC * (mc + 1))
                    nc.tensor.matmul(pv[:, ms], wc[0][:, 128 * mc:128 * (mc + 1)],
                                     xq[:, 0:BLOC], start=True, stop=False)
                    nc.tensor.matmul(pv[:, ms], wc[1][:, 128 * mc:128 * (mc + 1)],
                                     xq[:, BLOC:2 * BLOC], start=False, stop=True)
                for mc in range(2):
                    ms = slice(BLOC * mc, BLOC * (mc + 1))
                    nc.tensor.matmul(pu[:, ms], wa[0][:, 128 * mc:128 * (mc + 1)],
                                     xq[:, 0:BLOC], start=True, stop=False)
                    nc.tensor.matmul(pu[:, ms], wa[1][:, 128 * mc:128 * (mc + 1)],
                                     xq[:, BLOC:2 * BLOC], start=False, stop=True)

                # P = M + STEP*U  (off the tanh chain; runs while ACT computes tanh)
                p = ppool.tile([128, 2 * BLOC], F32, tag="p")
                nc.vector.scalar_tensor_tensor(
                    p[:], pu[:], STEP, m[:], op0=ALU.mult, op1=ALU.add)

                # tanh per m-chunk (fused per-partition bias), staggered so the
                # next-step k0 matmuls can start as soon as xq chunk0 lands
                tt = tpool.tile([128, 2 * BLOC], F32, tag="tt")
                nc.scalar.activation(tt[:, 0:BLOC], pv[:, 0:BLOC], AF.Tanh,
                                     bias=by_sb[:, 0:1], scale=1.0)
                nc.scalar.activation(tt[:, BLOC:2 * BLOC], pv[:, BLOC:2 * BLOC],
                                     AF.Tanh, bias=by_sb[:, 1:2], scale=1.0)

                # chain ops: next PE input (fp16), per chunk
                xq = xqpool.tile([128, 2 * BLOC], F16, tag="xq")
                nc.vector.scalar_tensor_tensor(
                    xq[:, 0:BLOC], tt[:, 0:BLOC], STEP, p[:, 0:BLOC],
                    op0=ALU.mult, op1=ALU.add)
                nc.vector.scalar_tensor_tensor(
                    xq[:, BLOC:2 * BLOC], tt[:, BLOC:2 * BLOC], STEP,
                    p[:, BLOC:2 * BLOC], op0=ALU.mult, op1=ALU.add)
                # master state, same math in fp32 (off chain)
                m = mpool.tile([128, 2 * BLOC], F32, tag="m")
                nc.vector.scalar_tensor_tensor(
                    m[:], tt[:], STEP, p[:], op0=ALU.mult, op1=ALU.add)

                # output row t: transpose state copy [128, 2b] -> [b, 256]
                pt = pst.tile([BLOC, N], F16, tag="pt")
                nc.tensor.transpose(pt[:, 0:128], xq[:, 0:BLOC], ident[:])
                nc.tensor.transpose(pt[:, 128:256], xq[:, BLOC:2 * BLOC], ident[:])
                stage = stpool.tile([BLOC, N], F32, tag="stage")
                nc.vector.tensor_copy(stage[:], pt[:])
                nc.sync.dma_start(OUT[:, t, :], stage[:])
    nc.compile()
    return nc


def kernel(X0, MA, MW, bA_z, bW_z, by_w):
    global LAST_RESULT
    from concourse.bass_utils import run_bass_kernel_spmd

    X0 = np.asarray(X0, dtype=np.float32)
    MA = np.asarray(MA, dtype=np.float32)
    MW = np.asarray(MW, dtype=np.float32)
    bA_z = np.asarray(bA_z, dtype=np.float32)
    bW_z = np.asarray(bW_z, dtype=np.float32)
    by_w = np.asarray(by_w, dtype=np.float32)

    # host-side weight prep (f32, matches reference math); weights to fp16
    # for full-rate PE matmuls (master state stays fp32 on device).
    bA = np.float32(0.5) * np.exp(-bA_z[0, 0] * bA_z[0, 0]) + np.float32(0.5)
    bW = np.float32(0.5) * np.exp(-bW_z[0, 0] * bW_z[0, 0]) + np.float32(0.5)
    I = np.eye(N, dtype=np.float32)
    A = (1 - bA) * (MA + MA.T) + bA * (MA - MA.T) - np.float32(YA) * I
    C = (1 - bA) * (MW + MW.T) + bW * (MW - MW.T) - np.float32(YW) * I
    WA = np.ascontiguousarray(A.T).astype(np.float16)
    WC = np.ascontiguousarray(C.T).astype(np.float16)

    n_steps = TMAX - 1
    in_maps = []
    for i in range(NCORES):
        in_maps.append({
            "WA": WA,
            "WC": WC,
            "BY": by_w,
            "X0T": np.ascontiguousarray(X0[i * BLOC:(i + 1) * BLOC, :].T),
        })

    nc = _build(n_steps)
    res = run_bass_kernel_spmd(nc, in_maps, core_ids=list(range(NCORES)))
    LAST_RESULT = res

    out = np.concatenate([r["OUT"] for r in res.results], axis=0)
    out[:, 0, :] = X0
    return out


if __name__ == "__main__":
    rng = np.random.default_rng(0)
    inputs = {
        "X0": rng.standard_normal((BS, N), dtype=np.float32),
        "MA": rng.standard_normal((N, N), dtype=np.float32) / 16,
        "MW": rng.standard_normal((N, N), dtype=np.float32) / 16,
        "bA_z": np.full((1, 1), 0.65, dtype=np.float32),
        "bW_z": np.full((1, 1), 0.65, dtype=np.float32),
        "by_w": rng.standard_normal((N, 1), dtype=np.float32) / 100,
    }
    out = kernel(**inputs)
    print("out", out.shape, out.dtype, np.abs(out).max())
